# revision 10
# baseline (speedup 1.0000x reference)
"""Trainium2 Bass kernel for nn_ARC_56461640073584 (sparse_attention).

Math (per batch b, head h):
  n = layernorm(x[b])                                  (parameter-free, per row over Din)
  seg(t) in {s: t<128, m: 128<=t<2176, e: t>=2176}
  q[t] = (n[t]*g_seg + b_seg) @ Wq_seg / 8   (k, v analogous, no /8)
  scoresT[t,s] = k[t].q[s] + (1 if t<=s else 0)
  att = softmax over t;  outT[h] = (v^T @ exp) / colsum(exp)

Device algorithm (per core: one batch, two heads):
  - inputs pre-laid-out on host (transpose/reshape only, no arithmetic)
  - LN folded into weights: Weff = g*W (on-device DVE fold), mean/std fixups as
    rank-1 matmul accumulations, 1/std applied at PSUM eviction.
  - stats via PE ones-matmuls over xT and xT^2
  - projections produce qT/kT/vT [2*64, T] with both heads packed in M
  - vT transposed to natural v via PE transpose; ones column appended for the
    softmax denominator (M=65 out matmul -> row 64 = denom)
  - scoresT tiles: K=64 matmuls row-packed per head (partitions 0-63 / 64-127)
  - mask (+1 lower triangle in [s,t] == t<=s) added via identity-matmul psum
    accumulate on diagonal tiles, exp bias=1.0 for fully-masked tiles
  - exp on ACT straight out of PSUM; out matmul accumulates exp @ [v|1]
  - normalize: PE broadcast of denom row, DVE reciprocal+mul, DMA out

Sharding: core c -> batch c//4, heads (2*(c%4), 2*(c%4)+1). Host gathers
[2,64,2304] per core into [2,2304,512].
"""

import numpy as np

B, T, DIN, H, DK = 2, 2304, 1024, 8, 64
L = 128           # state length (start/end segments)
KC = DIN // 128   # 8 contraction chunks
NT = T // 128     # 18 t tiles
SEG_BOUNDS = [(0, L, 0), (L, T - L, 1), (T - L, T, 2)]
# s-pieces for projections, aligned to segment boundaries, <=512 wide
PROJ_PIECES = []
for lo, hi, sg in SEG_BOUNDS:
    p = lo
    while p < hi:
        q = min(p + 512, hi)
        PROJ_PIECES.append((p, q, sg))
        p = q
# attention s blocks
S_BLOCKS = [(i * 512, min((i + 1) * 512, T)) for i in range((T + 511) // 512)]

_CACHE = {}


def _build():
    import concourse.bacc as bacc
    import concourse.tile as tile
    from concourse import mybir
    from concourse.masks import make_identity

    f32 = mybir.dt.float32
    f32r = mybir.dt.float32r
    AF = mybir.ActivationFunctionType
    OP = mybir.AluOpType

    nc = bacc.Bacc("TRN2", target_bir_lowering=False, debug=False)
    xT_d = nc.dram_tensor("xT", (128, KC, T), f32r, kind="ExternalInput")
    W_d = nc.dram_tensor("Wp", (128, 3, 3, KC, 128), f32r, kind="ExternalInput")
    G_d = nc.dram_tensor("Gp", (128, 3, 2, KC), f32, kind="ExternalInput")
    B_d = nc.dram_tensor("Bp", (128, 3, KC, 2), f32r, kind="ExternalInput")
    M_d = nc.dram_tensor("Mp", (128, 4, 512), f32r, kind="ExternalInput")
    out_d = nc.dram_tensor("out", (2, 64, T), f32, kind="ExternalOutput")

    with tile.TileContext(nc) as tc:
        with tc.tile_pool(name="consts", bufs=1) as consts:
            ident_f = consts.tile([128, 128], f32)
            make_identity(nc, ident_f)
            ident = consts.tile([128, 128], f32r)
            nc.vector.tensor_copy(ident, ident_f)
            ones_f = consts.tile([128, 128], f32)
            nc.vector.memset(ones_f, 1.0)
            ones_col = consts.tile([128, 1], f32r)
            nc.vector.tensor_copy(ones_col, ones_f[:, 0:1])
            # rows of ones at partition 0 (rstd bcast) and 64 (denom bcast)
            ones_r = consts.tile([128, 128], f32r)
            nc.vector.tensor_copy(ones_r[0:1, :], ones_f[0:1, :])
            nc.vector.tensor_copy(ones_r[64:65, :], ones_f[64:65, :])

            G_t = consts.tile([128, 3, 2, KC], f32)
            nc.sync.dma_start(G_t[:], G_d[:])
            Bb_t = consts.tile([128, 3, KC, 2], f32r)
            nc.sync.dma_start(Bb_t[:], B_d[:])
            cs_row = consts.tile([1, 3, 3, 128], f32r)   # colsum rows (proj, seg)
            b_row = consts.tile([1, 3, 3, 128], f32r)    # bias rows (proj, seg)
            # long-lived stats rows
            negm = consts.tile([1, T], f32r)
            stdr = consts.tile([1, T], f32r)
            rstdB = consts.tile([128, T], f32)
            # projections output (heads packed on partitions: h0 0-63, h1 64-127)
            qT = consts.tile([128, T], f32r)
            kT = consts.tile([128, T], f32r)
            vT = consts.tile([128, T], f32r)

            with tc.tile_pool(name="phase_a", bufs=1) as pha, \
                 tc.tile_pool(name="x2p", bufs=2) as x2p, \
                 tc.tile_pool(name="rowp", bufs=1) as rowp, \
                 tc.tile_pool(name="ps_st", bufs=2, space="PSUM") as ps_st, \
                 tc.tile_pool(name="ps_sm", bufs=2, space="PSUM") as ps_sm:
                xT = pha.tile([128, KC, T], f32r)
                nc.sync.dma_start(xT[:], xT_d[:])
                W_t = pha.tile([128, 3, 3, KC, 128], f32r)
                nc.sync.dma_start(W_t[:], W_d[:])

                # ---- bias rows: b_seg @ W  (uses RAW W, before the g-fold) ----
                for p in range(3):
                    for sg in range(3):
                        pb = ps_sm.tile([1, 256], f32, tag="small")
                        for h in range(2):
                            for kc in range(KC):
                                nc.tensor.matmul(
                                    pb[0:1, 128 * h:128 * h + 128],
                                    Bb_t[:, sg, kc, h:h + 1],
                                    W_t[:, p, sg, kc, :],
                                    start=(kc == 0), stop=(kc == KC - 1),
                                )
                        sc = 0.125 if p == 0 else 1.0
                        nc.vector.tensor_scalar_mul(
                            b_row[0:1, p, sg, 0:64], pb[0:1, 0:64], sc)
                        nc.vector.tensor_scalar_mul(
                            b_row[0:1, p, sg, 64:128], pb[0:1, 192:256], sc)

                # ---- fold Weff = g * W (in place), q scaled by 1/8 ----
                for p in range(3):
                    for sg in range(3):
                        for kc in range(KC):
                            for h in range(2):
                                w_sl = W_t[:, p, sg, kc, 64 * h:64 * h + 64]
                                g_sl = G_t[:, sg, h, kc:kc + 1]
                                if p == 0:
                                    nc.vector.tensor_scalar(
                                        w_sl, w_sl, g_sl, 0.125,
                                        op0=OP.mult, op1=OP.mult)
                                else:
                                    nc.vector.tensor_scalar_mul(w_sl, w_sl, g_sl)

                # ---- colsum rows of Weff ----
                for p in range(3):
                    for sg in range(3):
                        pc = ps_sm.tile([1, 256], f32, tag="small")
                        for kc in range(KC):
                            nc.tensor.matmul(
                                pc[0:1, 0:128], ones_col, W_t[:, p, sg, kc, :],
                                start=(kc == 0), stop=(kc == KC - 1))
                        nc.vector.tensor_copy(cs_row[0:1, p, sg, :],
                                              pc[0:1, 0:128])

                # ---- stats: sum x -> negm tile, sum x^2 -> stdr tile ----
                for lo, hi in S_BLOCKS:
                    ln = hi - lo
                    p1 = ps_st.tile([1, 512], f32, tag="st")
                    p2 = ps_st.tile([1, 512], f32, tag="st")
                    for kc in range(KC):
                        x2 = x2p.tile([128, 512], f32r, tag="x2")
                        nc.vector.tensor_mul(
                            x2[:, :ln], xT[:, kc, lo:hi], xT[:, kc, lo:hi])
                        nc.tensor.matmul(p1[0:1, :ln], ones_col,
                                         xT[:, kc, lo:hi],
                                         start=(kc == 0), stop=(kc == KC - 1))
                        nc.tensor.matmul(p2[0:1, :ln], ones_col,
                                         x2[:, :ln],
                                         start=(kc == 0), stop=(kc == KC - 1))
                    nc.vector.tensor_copy(negm[0:1, lo:hi], p1[0:1, :ln])
                    nc.vector.tensor_copy(stdr[0:1, lo:hi], p2[0:1, :ln])

                # negm = -sumx/D ; var+eps = sumx2/D - m^2 + eps ; std ; rstd
                m2 = rowp.tile([1, T], f32, tag="m2")
                nc.vector.tensor_scalar_mul(negm[0:1, :], negm[0:1, :],
                                            -1.0 / DIN)
                nc.vector.tensor_mul(m2[0:1, :], negm[0:1, :], negm[0:1, :])
                nc.vector.tensor_scalar(stdr[0:1, :], stdr[0:1, :],
                                        1.0 / DIN, 1e-5, op0=OP.mult, op1=OP.add)
                nc.vector.tensor_sub(stdr[0:1, :], stdr[0:1, :], m2[0:1, :])
                nc.scalar.activation(stdr[0:1, :], stdr[0:1, :], AF.Sqrt)
                rstd = rowp.tile([1, T], f32r, tag="rstd")
                with nc.allow_low_precision(reason="fp32r rstd, 6e-5 rel"):
                    nc.vector.reciprocal(rstd[0:1, :], stdr[0:1, :])

                # rstdB: broadcast rstd across 128 partitions via PE
                for lo, hi in S_BLOCKS:
                    ln = hi - lo
                    pr = ps_st.tile([128, 512], f32, tag="rB")
                    nc.tensor.matmul(pr[:, :ln], ones_r[0:1, :],
                                     rstd[0:1, lo:hi], start=True, stop=True)
                    nc.vector.tensor_copy(rstdB[:, lo:hi], pr[:, :ln])

                # ---- projections: psum = Weff^T @ xT - colsum x m + brow x std
                #      then eviction multiplies by rstd (broadcast) ----
                with tc.tile_pool(name="ps_pr", bufs=2, space="PSUM") as ps_pr:
                    for p, dst in ((0, qT), (1, kT), (2, vT)):
                        for lo, hi, sg in PROJ_PIECES:
                            ln = hi - lo
                            pp = ps_pr.tile([128, 512], f32, tag="proj")
                            for kc in range(KC):
                                nc.tensor.matmul(
                                    pp[:, :ln],
                                    W_t[:, p, sg, kc, :],
                                    xT[:, kc, lo:hi],
                                    start=(kc == 0), stop=False)
                            nc.tensor.matmul(pp[:, :ln], cs_row[0:1, p, sg, :],
                                             negm[0:1, lo:hi],
                                             start=False, stop=False)
                            nc.tensor.matmul(pp[:, :ln], b_row[0:1, p, sg, :],
                                             stdr[0:1, lo:hi],
                                             start=False, stop=True)
                            nc.vector.tensor_mul(dst[:, lo:hi], pp[:, :ln],
                                                 rstdB[:, lo:hi])

            # ---- phase B: v transpose + attention ----
            phb_cm = tc.tile_pool(name="phase_b", bufs=1)
            phb = phb_cm.__enter__()
            vaug = phb.tile([128, 2, NT, 65], f32r)
            for _h in range(2):
                nc.vector.tensor_copy(vaug[:, _h, :, 64], ones_f[:, 0:NT])
            mask = phb.tile([128, 4, 512], f32r)
            nc.sync.dma_start(mask[:], M_d[:])

            with tc.tile_pool(name="ps_vt", bufs=2, space="PSUM") as ps_vt:
                for h in range(2):
                    for g4 in range((NT + 3) // 4):  # 4 transposes per bank
                        n4 = min(4, NT - 4 * g4)
                        pv = ps_vt.tile([128, 4, 64], f32r, tag="vt")
                        for j in range(n4):
                            tt = 4 * g4 + j
                            hp = 64 * h
                            nc.tensor.transpose(
                                pv[:, j, :],
                                vT[hp:hp + 64, 128 * tt:128 * tt + 128],
                                ident[hp:hp + 64, hp:hp + 64])
                        nc.vector.tensor_copy(
                            vaug[:, h, 4 * g4:4 * g4 + n4, 0:64],
                            pv[:, 0:n4, :])

            with tc.tile_pool(name="ps_sc", bufs=3, space="PSUM") as ps_sc, \
                 tc.tile_pool(name="ps_out", bufs=3, space="PSUM") as ps_out, \
                 tc.tile_pool(name="ps_rd", bufs=2, space="PSUM") as ps_rd, \
                 tc.tile_pool(name="expp", bufs=3) as expp, \
                 tc.tile_pool(name="outp", bufs=3) as outp:
                for lo, hi in S_BLOCKS:
                    ln = hi - lo
                    po = []
                    for _h in range(2):
                        po_t = ps_out.tile([65, 512], f32, tag="out")
                        po.append(po_t)
                    for tt in range(NT):
                        jdiag = (tt * 128) // 512
                        jme = lo // 512
                        for h in range(2):
                            hp = 64 * h
                            ps = ps_sc.tile([128, 512], f32, tag="sc")
                            diag = jme == jdiag
                            nc.tensor.matmul(
                                ps[:, :ln],
                                kT[hp:hp + 64, 128 * tt:128 * tt + 128],
                                qT[hp:hp + 64, lo:hi],
                                start=True, stop=not diag)
                            bias = 1.0 if jme > jdiag else 0.0
                            if diag:
                                off = tt * 128 - 512 * jdiag
                                nc.tensor.matmul(
                                    ps[:, :ln], ident,
                                    mask[:, off // 128, 0:ln],
                                    start=False, stop=True)
                                bias = 0.0
                            ex = expp.tile([128, 512], f32r, tag=f"ex{h}")
                            nc.scalar.activation(ex[:, :ln], ps[:, :ln],
                                                 AF.Exp, bias=bias)
                            nc.tensor.matmul(
                                po[h][:, :ln],
                                vaug[:, h, tt, :],
                                ex[:, :ln],
                                start=(tt == 0), stop=(tt == NT - 1))
                    # normalize + store
                    for h in range(2):
                        den = outp.tile([128, 512], f32r, tag="den")
                        nc.vector.tensor_copy(den[64:65, :ln],
                                              po[h][64:65, :ln])
                        prd = ps_rd.tile([64, 512], f32, tag="rd")
                        nc.tensor.matmul(prd[:, :ln], ones_r[64:65, 0:64],
                                         den[64:65, :ln], start=True, stop=True)
                        rd = outp.tile([64, 512], f32, tag="rd_s")
                        nc.vector.reciprocal(rd[:, :ln], prd[:, :ln])
                        ot = outp.tile([64, 512], f32, tag="ot")
                        nc.vector.tensor_mul(ot[:, :ln], po[h][0:64, :ln],
                                             rd[:, :ln])
                        nc.sync.dma_start(out_d[h, :, lo:hi], ot[:, :ln])
            phb_cm.__exit__(None, None, None)
    nc.finalize()
    return nc


def _prep_inputs(x, ln_g, ln_b, ln_gs, ln_bs, ln_ge, ln_be,
                 Wq, Wk, Wv, Wq_s, Wk_s, Wv_s, Wq_e, Wk_e, Wv_e):
    """Per-core input dicts. Layout/reshape only — all arithmetic on device."""
    gmap = [(ln_gs, ln_bs), (ln_g, ln_b), (ln_ge, ln_be)]
    wmap = [(Wq_s, Wk_s, Wv_s), (Wq, Wk, Wv), (Wq_e, Wk_e, Wv_e)]

    # causal-diagonal mask tiles: mk[o][r, c] = 1 if c >= 128*o + r
    rr = np.arange(128)[:, None]
    cc = np.arange(512)[None, :]
    mk = np.stack([(cc >= (128 * o + rr)) for o in range(4)], 0).astype(np.float32)
    Mp = np.ascontiguousarray(mk.transpose(1, 0, 2))  # [128, 4, 512]

    maps = []
    for c in range(8):
        b = c // 4
        h0 = 2 * (c % 4)
        xT = np.ascontiguousarray(
            x[b].T.reshape(KC, 128, T).transpose(1, 0, 2))  # [128, KC, T]
        Wp = np.empty((128, 3, 3, KC, 128), np.float32)
        Gp = np.empty((128, 3, 2, KC), np.float32)
        Bp = np.empty((128, 3, KC, 2), np.float32)
        for sg in range(3):
            g3, b3 = gmap[sg]
            for j, wmat in enumerate(wmap[sg]):
                for h in range(2):
                    # [KC,128,64] view of W[head][1024, 64]
                    wv = wmat[h0 + h].reshape(KC, 128, DK)
                    Wp[:, j, sg, :, 64 * h:64 * h + 64] = wv.transpose(1, 0, 2)
            for h in range(2):
                Gp[:, sg, h, :] = g3[h0 + h].reshape(KC, 128).T
                Bp[:, sg, :, h] = b3[h0 + h].reshape(KC, 128).T
        maps.append({
            "xT": xT, "Wp": np.ascontiguousarray(Wp),
            "Gp": np.ascontiguousarray(Gp), "Bp": np.ascontiguousarray(Bp),
            "Mp": Mp,
        })
    return maps


def kernel(**inputs):
    from concourse.bass_utils import run_bass_kernel_spmd

    offset = inputs.pop("offset", None)  # unused by the reference config
    if "nc" not in _CACHE:
        _CACHE["nc"] = _build()
    nc = _CACHE["nc"]
    in_maps = _prep_inputs(**{k: np.asarray(v, np.float32)
                              for k, v in inputs.items()})
    trace = _CACHE.get("trace", False)
    res = run_bass_kernel_spmd(nc, in_maps, core_ids=list(range(8)),
                               trace=trace)
    _CACHE["last_result"] = res
    out = np.empty((B, T, H * DK), np.float32)
    for c in range(8):
        b = c // 4
        h0 = 2 * (c % 4)
        oc = res.results[c]["out"]  # [2, 64, T]
        for j in range(2):
            out[b, :, 64 * (h0 + j):64 * (h0 + j) + 64] = oc[j].T
    return out


# revision 11
# speedup vs baseline: 1.3220x; 1.3220x over previous
"""Trainium2 Bass kernel for nn_ARC_56461640073584 (sparse_attention).

Math (per batch b, head h):
  n = layernorm(x[b])                                  (parameter-free, per row over Din)
  seg(t) in {s: t<128, m: 128<=t<2176, e: t>=2176}
  q[t] = (n[t]*g_seg + b_seg) @ Wq_seg / 8   (k, v analogous, no /8)
  scoresT[t,s] = k[t].q[s] + (1 if t<=s else 0)
  att = softmax over t;  outT[h] = (v^T @ exp) / colsum(exp)

Device algorithm (per core: one batch, two heads):
  - inputs pre-laid-out on host (transpose/reshape only, no arithmetic)
  - LN folded into weights: Weff = g*W (on-device DVE fold), mean/std fixups as
    rank-1 matmul accumulations, 1/std applied at PSUM eviction.
  - stats via PE ones-matmuls over xT and xT^2
  - projections produce qT/kT/vT [2*64, T] with both heads packed in M
  - vT transposed to natural v via PE transpose; ones column appended for the
    softmax denominator (M=65 out matmul -> row 64 = denom)
  - scoresT tiles: K=64 matmuls row-packed per head (partitions 0-63 / 64-127)
  - mask (+1 lower triangle in [s,t] == t<=s) added via identity-matmul psum
    accumulate on diagonal tiles, exp bias=1.0 for fully-masked tiles
  - exp on ACT straight out of PSUM; out matmul accumulates exp @ [v|1]
  - normalize: PE broadcast of denom row, DVE reciprocal+mul, DMA out

Sharding: core c -> batch c//4, heads (2*(c%4), 2*(c%4)+1). Host gathers
[2,64,2304] per core into [2,2304,512].
"""

import numpy as np

B, T, DIN, H, DK = 2, 2304, 1024, 8, 64
L = 128           # state length (start/end segments)
KC = DIN // 128   # 8 contraction chunks
NT = T // 128     # 18 t tiles
SEG_BOUNDS = [(0, L, 0), (L, T - L, 1), (T - L, T, 2)]
# s-pieces for projections, aligned to segment boundaries, <=512 wide
PROJ_PIECES = []
for lo, hi, sg in SEG_BOUNDS:
    p = lo
    while p < hi:
        q = min(p + 512, hi)
        PROJ_PIECES.append((p, q, sg))
        p = q
# attention s blocks
S_BLOCKS = [(i * 512, min((i + 1) * 512, T)) for i in range((T + 511) // 512)]

_CACHE = {}


def _build():
    import concourse.bacc as bacc
    import concourse.tile as tile
    from concourse import mybir
    from concourse.masks import make_identity

    f32 = mybir.dt.float32
    f32r = mybir.dt.float32r
    bf16 = mybir.dt.bfloat16
    AF = mybir.ActivationFunctionType
    OP = mybir.AluOpType

    nc = bacc.Bacc("TRN2", target_bir_lowering=False, debug=False)
    xT_d = nc.dram_tensor("xT", (128, KC, T), f32r, kind="ExternalInput")
    W_d = nc.dram_tensor("Wp", (128, 3, 3, KC, 128), f32r, kind="ExternalInput")
    G_d = nc.dram_tensor("Gp", (128, 3, 2, KC), f32, kind="ExternalInput")
    B_d = nc.dram_tensor("Bp", (128, 3, KC, 2), f32r, kind="ExternalInput")
    M_d = nc.dram_tensor("Mp", (128, 4, 512), f32r, kind="ExternalInput")
    out_d = nc.dram_tensor("out", (2, 64, T), f32, kind="ExternalOutput")

    with tile.TileContext(nc) as tc:
        with tc.tile_pool(name="consts", bufs=1) as consts:
            ident_f = consts.tile([128, 128], f32)
            make_identity(nc, ident_f)
            ident = consts.tile([128, 128], f32r)
            nc.vector.tensor_copy(ident, ident_f)
            ones_f = consts.tile([128, 128], f32)
            nc.vector.memset(ones_f, 1.0)
            ones_col = consts.tile([128, 1], f32r)
            nc.vector.tensor_copy(ones_col, ones_f[:, 0:1])
            # rows of ones at partition 0 (rstd bcast) and 64 (denom bcast)
            ones_r = consts.tile([128, 128], f32r)
            nc.vector.tensor_copy(ones_r[0:1, :], ones_f[0:1, :])
            nc.vector.tensor_copy(ones_r[64:65, :], ones_f[64:65, :])

            G_t = consts.tile([128, 3, 2, KC], f32)
            nc.sync.dma_start(G_t[:], G_d[:])
            Bb_t = consts.tile([128, 3, KC, 2], f32r)
            nc.sync.dma_start(Bb_t[:], B_d[:])
            cs_row = consts.tile([1, 3, 3, 128], f32r)   # colsum rows (proj, seg)
            b_row = consts.tile([1, 3, 3, 128], f32r)    # bias rows (proj, seg)
            # long-lived stats rows
            negm = consts.tile([1, T], f32r)
            stdr = consts.tile([1, T], f32r)
            rstdB = consts.tile([128, T], f32)
            # projections output (heads packed on partitions: h0 0-63, h1 64-127)
            qT = consts.tile([128, T], f32r)
            kT = consts.tile([128, T], f32r)
            vT = consts.tile([128, T], f32r)

            with tc.tile_pool(name="phase_a", bufs=1) as pha, \
                 tc.tile_pool(name="x2p", bufs=2) as x2p, \
                 tc.tile_pool(name="rowp", bufs=1) as rowp, \
                 tc.tile_pool(name="ps_st", bufs=2, space="PSUM") as ps_st, \
                 tc.tile_pool(name="ps_sm", bufs=2, space="PSUM") as ps_sm:
                xT = pha.tile([128, KC, T], f32r)
                for kc in range(KC):
                    nc.sync.dma_start(xT[:, kc, :], xT_d[:, kc, :])
                W_t = pha.tile([128, 3, 3, KC, 128], f32r)
                for p in range(3):
                    nc.sync.dma_start(W_t[:, p], W_d[:, p])

                # ---- bias rows: b_seg @ W  (uses RAW W, before the g-fold) ----
                for p in range(3):
                    for sg in range(3):
                        pb = ps_sm.tile([1, 256], f32, tag="small")
                        for h in range(2):
                            for kc in range(KC):
                                nc.tensor.matmul(
                                    pb[0:1, 128 * h:128 * h + 128],
                                    Bb_t[:, sg, kc, h:h + 1],
                                    W_t[:, p, sg, kc, :],
                                    start=(kc == 0), stop=(kc == KC - 1),
                                )
                        sc = 0.125 if p == 0 else 1.0
                        nc.vector.tensor_scalar_mul(
                            b_row[0:1, p, sg, 0:64], pb[0:1, 0:64], sc)
                        nc.vector.tensor_scalar_mul(
                            b_row[0:1, p, sg, 64:128], pb[0:1, 192:256], sc)

                # ---- fold Weff = g * W (in place), q scaled by 1/8 ----
                for p in range(3):
                    for sg in range(3):
                        for kc in range(KC):
                            for h in range(2):
                                w_sl = W_t[:, p, sg, kc, 64 * h:64 * h + 64]
                                g_sl = G_t[:, sg, h, kc:kc + 1]
                                if p == 0:
                                    nc.gpsimd.tensor_scalar(
                                        w_sl, w_sl, g_sl, 0.125,
                                        op0=OP.mult, op1=OP.mult)
                                else:
                                    nc.gpsimd.tensor_scalar_mul(w_sl, w_sl, g_sl)

                # ---- colsum rows of Weff ----
                for p in range(3):
                    for sg in range(3):
                        pc = ps_sm.tile([1, 256], f32, tag="small")
                        for kc in range(KC):
                            nc.tensor.matmul(
                                pc[0:1, 0:128], ones_col, W_t[:, p, sg, kc, :],
                                start=(kc == 0), stop=(kc == KC - 1))
                        nc.vector.tensor_copy(cs_row[0:1, p, sg, :],
                                              pc[0:1, 0:128])

                # ---- stats: sum x -> negm tile, sum x^2 -> stdr tile ----
                for lo, hi in S_BLOCKS:
                    ln = hi - lo
                    p1 = ps_st.tile([1, 512], f32, tag="st")
                    p2 = ps_st.tile([1, 512], f32, tag="st")
                    for kc in range(KC):
                        x2 = x2p.tile([128, 512], f32r, tag="x2")
                        nc.gpsimd.tensor_mul(
                            x2[:, :ln], xT[:, kc, lo:hi], xT[:, kc, lo:hi])
                        nc.tensor.matmul(p1[0:1, :ln], ones_col,
                                         xT[:, kc, lo:hi],
                                         start=(kc == 0), stop=(kc == KC - 1))
                        nc.tensor.matmul(p2[0:1, :ln], ones_col,
                                         x2[:, :ln],
                                         start=(kc == 0), stop=(kc == KC - 1))
                    nc.vector.tensor_copy(negm[0:1, lo:hi], p1[0:1, :ln])
                    nc.vector.tensor_copy(stdr[0:1, lo:hi], p2[0:1, :ln])

                # negm = -sumx/D ; var+eps = sumx2/D - m^2 + eps ; std ; rstd
                m2 = rowp.tile([1, T], f32, tag="m2")
                nc.vector.tensor_scalar_mul(negm[0:1, :], negm[0:1, :],
                                            -1.0 / DIN)
                nc.vector.tensor_mul(m2[0:1, :], negm[0:1, :], negm[0:1, :])
                nc.vector.tensor_scalar(stdr[0:1, :], stdr[0:1, :],
                                        1.0 / DIN, 1e-5, op0=OP.mult, op1=OP.add)
                nc.vector.tensor_sub(stdr[0:1, :], stdr[0:1, :], m2[0:1, :])
                nc.scalar.activation(stdr[0:1, :], stdr[0:1, :], AF.Sqrt)
                rstd = rowp.tile([1, T], f32r, tag="rstd")
                with nc.allow_low_precision(reason="fp32r rstd, 6e-5 rel"):
                    nc.vector.reciprocal(rstd[0:1, :], stdr[0:1, :])

                # rstdB: broadcast rstd across 128 partitions via PE
                for lo, hi in S_BLOCKS:
                    ln = hi - lo
                    pr = ps_st.tile([128, 512], f32, tag="rB")
                    nc.tensor.matmul(pr[:, :ln], ones_r[0:1, :],
                                     rstd[0:1, lo:hi], start=True, stop=True)
                    nc.vector.tensor_copy(rstdB[:, lo:hi], pr[:, :ln])

                # ---- projections: psum = Weff^T @ xT - colsum x m + brow x std
                #      then eviction multiplies by rstd (broadcast) ----
                with tc.tile_pool(name="ps_pr", bufs=2, space="PSUM") as ps_pr:
                    for p, dst in ((0, qT), (1, kT), (2, vT)):
                        for lo, hi, sg in PROJ_PIECES:
                            ln = hi - lo
                            pp = ps_pr.tile([128, 512], f32, tag="proj")
                            for kc in range(KC):
                                nc.tensor.matmul(
                                    pp[:, :ln],
                                    W_t[:, p, sg, kc, :],
                                    xT[:, kc, lo:hi],
                                    start=(kc == 0), stop=False)
                            nc.tensor.matmul(pp[:, :ln], cs_row[0:1, p, sg, :],
                                             negm[0:1, lo:hi],
                                             start=False, stop=False)
                            nc.tensor.matmul(pp[:, :ln], b_row[0:1, p, sg, :],
                                             stdr[0:1, lo:hi],
                                             start=False, stop=True)
                            nc.vector.tensor_mul(dst[:, lo:hi], pp[:, :ln],
                                                 rstdB[:, lo:hi])

            # ---- phase B: v transpose + attention ----
            phb_cm = tc.tile_pool(name="phase_b", bufs=1)
            phb = phb_cm.__enter__()
            vaug = phb.tile([128, 2, NT, 65], bf16)
            for _h in range(2):
                nc.vector.tensor_copy(vaug[:, _h, :, 64], ones_f[:, 0:NT])
            mask = phb.tile([128, 4, 512], f32r)
            nc.sync.dma_start(mask[:], M_d[:])

            with tc.tile_pool(name="ps_vt", bufs=2, space="PSUM") as ps_vt:
                for h in range(2):
                    for g4 in range((NT + 3) // 4):  # 4 transposes per bank
                        n4 = min(4, NT - 4 * g4)
                        pv = ps_vt.tile([128, 4, 64], f32r, tag="vt")
                        for j in range(n4):
                            tt = 4 * g4 + j
                            hp = 64 * h
                            nc.tensor.transpose(
                                pv[:, j, :],
                                vT[hp:hp + 64, 128 * tt:128 * tt + 128],
                                ident[hp:hp + 64, hp:hp + 64])
                        nc.vector.tensor_copy(
                            vaug[:, h, 4 * g4:4 * g4 + n4, 0:64],
                            pv[:, 0:n4, :])

            with tc.tile_pool(name="ps_sc", bufs=3, space="PSUM") as ps_sc, \
                 tc.tile_pool(name="ps_out", bufs=3, space="PSUM") as ps_out, \
                 tc.tile_pool(name="ps_rd", bufs=2, space="PSUM") as ps_rd, \
                 tc.tile_pool(name="expp", bufs=3) as expp, \
                 tc.tile_pool(name="outp", bufs=3) as outp:
                for lo, hi in S_BLOCKS:
                    ln = hi - lo
                    po = []
                    for _h in range(2):
                        po_t = ps_out.tile([65, 512], f32, tag="out")
                        po.append(po_t)
                    for tt in range(NT):
                        jdiag = (tt * 128) // 512
                        jme = lo // 512
                        for h in range(2):
                            hp = 64 * h
                            ps = ps_sc.tile([128, 512], f32, tag="sc")
                            diag = jme == jdiag
                            nc.tensor.matmul(
                                ps[:, :ln],
                                kT[hp:hp + 64, 128 * tt:128 * tt + 128],
                                qT[hp:hp + 64, lo:hi],
                                start=True, stop=not diag)
                            bias = 1.0 if jme > jdiag else 0.0
                            if diag:
                                off = tt * 128 - 512 * jdiag
                                nc.tensor.matmul(
                                    ps[:, :ln], ident,
                                    mask[:, off // 128, 0:ln],
                                    start=False, stop=True)
                                bias = 0.0
                            ex = expp.tile([128, 512], bf16, tag=f"ex{h}")
                            nc.scalar.activation(ex[:, :ln], ps[:, :ln],
                                                 AF.Exp, bias=bias)
                            nc.tensor.matmul(
                                po[h][:, :ln],
                                vaug[:, h, tt, :],
                                ex[:, :ln],
                                start=(tt == 0), stop=(tt == NT - 1))
                    # normalize + store
                    for h in range(2):
                        den = outp.tile([128, 512], f32r, tag="den")
                        nc.vector.tensor_copy(den[64:65, :ln],
                                              po[h][64:65, :ln])
                        prd = ps_rd.tile([64, 512], f32, tag="rd")
                        nc.tensor.matmul(prd[:, :ln], ones_r[64:65, 0:64],
                                         den[64:65, :ln], start=True, stop=True)
                        rd = outp.tile([64, 512], f32, tag="rd_s")
                        nc.vector.reciprocal(rd[:, :ln], prd[:, :ln])
                        ot = outp.tile([64, 512], f32, tag="ot")
                        nc.vector.tensor_mul(ot[:, :ln], po[h][0:64, :ln],
                                             rd[:, :ln])
                        nc.sync.dma_start(out_d[h, :, lo:hi], ot[:, :ln])
            phb_cm.__exit__(None, None, None)
    nc.finalize()
    return nc


def _prep_inputs(x, ln_g, ln_b, ln_gs, ln_bs, ln_ge, ln_be,
                 Wq, Wk, Wv, Wq_s, Wk_s, Wv_s, Wq_e, Wk_e, Wv_e):
    """Per-core input dicts. Layout/reshape only — all arithmetic on device."""
    gmap = [(ln_gs, ln_bs), (ln_g, ln_b), (ln_ge, ln_be)]
    wmap = [(Wq_s, Wk_s, Wv_s), (Wq, Wk, Wv), (Wq_e, Wk_e, Wv_e)]

    # causal-diagonal mask tiles: mk[o][r, c] = 1 if c >= 128*o + r
    rr = np.arange(128)[:, None]
    cc = np.arange(512)[None, :]
    mk = np.stack([(cc >= (128 * o + rr)) for o in range(4)], 0).astype(np.float32)
    Mp = np.ascontiguousarray(mk.transpose(1, 0, 2))  # [128, 4, 512]

    maps = []
    for c in range(8):
        b = c // 4
        h0 = 2 * (c % 4)
        xT = np.ascontiguousarray(
            x[b].T.reshape(KC, 128, T).transpose(1, 0, 2))  # [128, KC, T]
        Wp = np.empty((128, 3, 3, KC, 128), np.float32)
        Gp = np.empty((128, 3, 2, KC), np.float32)
        Bp = np.empty((128, 3, KC, 2), np.float32)
        for sg in range(3):
            g3, b3 = gmap[sg]
            for j, wmat in enumerate(wmap[sg]):
                for h in range(2):
                    # [KC,128,64] view of W[head][1024, 64]
                    wv = wmat[h0 + h].reshape(KC, 128, DK)
                    Wp[:, j, sg, :, 64 * h:64 * h + 64] = wv.transpose(1, 0, 2)
            for h in range(2):
                Gp[:, sg, h, :] = g3[h0 + h].reshape(KC, 128).T
                Bp[:, sg, :, h] = b3[h0 + h].reshape(KC, 128).T
        maps.append({
            "xT": xT, "Wp": np.ascontiguousarray(Wp),
            "Gp": np.ascontiguousarray(Gp), "Bp": np.ascontiguousarray(Bp),
            "Mp": Mp,
        })
    return maps


def kernel(**inputs):
    from concourse.bass_utils import run_bass_kernel_spmd

    offset = inputs.pop("offset", None)  # unused by the reference config
    if "nc" not in _CACHE:
        _CACHE["nc"] = _build()
    nc = _CACHE["nc"]
    in_maps = _prep_inputs(**{k: np.asarray(v, np.float32)
                              for k, v in inputs.items()})
    trace = _CACHE.get("trace", False)
    res = run_bass_kernel_spmd(nc, in_maps, core_ids=list(range(8)),
                               trace=trace)
    _CACHE["last_result"] = res
    out = np.empty((B, T, H * DK), np.float32)
    for c in range(8):
        b = c // 4
        h0 = 2 * (c % 4)
        oc = res.results[c]["out"]  # [2, 64, T]
        for j in range(2):
            out[b, :, 64 * (h0 + j):64 * (h0 + j) + 64] = oc[j].T
    return out
